# revision 49
# baseline (speedup 1.0000x reference)
"""Causal single-head attention (B=4, T=2048, E=1024, H=128) on 8 NeuronCores.

Sharding: core = (batch b, parity h). Block-cyclic over query blocks: core h of
batch b owns q-blocks {h, h+2, ..., h+14} (8 blocks of 128 rows), so causal
score work is near-balanced (64 vs 72 key-block units) instead of the 1:3 skew
of a contiguous split. Stage s handles local q-block s (global 2s+h) and needs
key slots 0..2s+1.

Per-core SBUF x layout puts each pair of key blocks {2s+h, 2s+1-h} with the
core's OWN block first, so the SPMD program is identical across cores; the
causal mask for the last two key slots is per-core DATA ([tri|-BIG] on even
cores, [tri|0] on odd).

Everything is fp16 on the PE (1 cycle/row at any tile size, halves HBM bytes);
accumulation stays fp32 in PSUM. Per stage:
  kT[:, 2s*128:+256]  = Wk^T x  (PE, W stationary, x moving) + bias  (DVE, fp16)
  qT[:, s*128:+128]   = Wq^T x own block (+ bias, pre-scaled 1/sqrt(H); ACT)
  v natural           = x^T Wv (PE, x stationary -> no transposes), copied to
                        vaug [pos, H | 1] (DVE, fp16)
  scores^T chunks     = kT_slot^T @ qT_s  (PE, [keys,128q] per slot, 4/bank)
  exp                 (ACT, fp16; no max-subtraction: scores ~ N(0,1))
  out|denom           = sum_kb expS_kb^T @ vaug_kb  (PE, fp32 PSUM)
  y = out * (1/denom) (DVE), DMA out per q-block.

Emission runs projections three stages ahead of attention (all chunk scores
before any AV, final AV chunk deferred past the next projection) so PE always
has work covering exp latency; the two largest attention stages interleave at
the tail. DMA: consts on the gpsimd queue, the first two x pairs on the
scalar queue (parallel DGE spin-up, alternating with weights on sync), so the
first k-projection starts ~3us in. bv is added on the host after gather
(softmax rows sum to 1, so +bv commutes).
"""

import math

import numpy as np

import concourse.tile as tile
from concourse import bacc, mybir
from concourse.bass_utils import run_bass_kernel_spmd

B, T, E, H = 4, 2048, 1024, 128
NB = T // 128        # 16 key slots
NE = E // 128        # 8 contraction chunks
NS = 8               # stages (local q-blocks) per core
BIG = 1.0e30

F32 = mybir.dt.float32
F16 = mybir.dt.float16

_CACHE: dict = {}


def _build():
    nc = bacc.Bacc(None, target_bir_lowering=False)
    xd = nc.dram_tensor("xd", [128, NE * T], F16, kind="ExternalInput")
    wd = nc.dram_tensor("wd", [128, 3 * E], F16, kind="ExternalInput")
    bqd = nc.dram_tensor("bqd", [128, 1], F32, kind="ExternalInput")
    bkd = nc.dram_tensor("bkd", [128, 1], F32, kind="ExternalInput")
    md = nc.dram_tensor("md", [128, 256], F32, kind="ExternalInput")
    y = nc.dram_tensor("y", [NS * 128, H], F32, kind="ExternalOutput")

    WK, WQ, WV = 0, E, 2 * E  # wsb column offsets

    with tile.TileContext(nc) as tc:
        with (
            tc.tile_pool(name="xs", bufs=1) as xs_pool,
            tc.tile_pool(name="ws", bufs=1) as w_pool,
            tc.tile_pool(name="small", bufs=1) as sm_pool,
            tc.tile_pool(name="qk", bufs=1) as qk_pool,
            tc.tile_pool(name="ex", bufs=8) as ex_pool,
            tc.tile_pool(name="ob", bufs=2) as ob_pool,
            tc.tile_pool(name="pp", bufs=2, space="PSUM") as pp_pool,
            tc.tile_pool(name="sps", bufs=4, space="PSUM") as sps_pool,
            tc.tile_pool(name="avp", bufs=2, space="PSUM") as av_pool,
        ):
            xs = xs_pool.tile([128, NE * T], F16)
            wsb = w_pool.tile([128, 3 * E], F16)
            bq_sb = sm_pool.tile([128, 1], F32, tag="bq")
            bk_sb = sm_pool.tile([128, 1], F32, tag="bk")
            mask_sb = sm_pool.tile([128, 256], F32, tag="msk")
            kT = qk_pool.tile([128, T], F16, tag="kT")
            qT = qk_pool.tile([128, NS * 128], F16, tag="qT")
            vaug = qk_pool.tile([128, NB * 129], F16, tag="vaug")

            # small consts on the gpsimd (SWDGE) queue, mask first since it
            # gates the earliest stages' exps; first two x pairs on the
            # scalar queue so its DGE pipeline spins up in parallel with
            # sync's; weights + remaining pairs on sync.
            nc.gpsimd.dma_start(mask_sb[:], md[:])
            nc.gpsimd.dma_start(bk_sb[:], bkd[:])
            nc.gpsimd.dma_start(bq_sb[:], bqd[:])
            nc.vector.memset(vaug[:], 1.0)  # ones column for denominators
            for c in range(4):  # first two pairs in halves on the scalar
                nc.scalar.dma_start(                # queue, alternating with
                    xs[:, c * 1024:(c + 1) * 1024],  # weights on sync
                    xd[:, c * 1024:(c + 1) * 1024])
            nc.sync.dma_start(wsb[:, WK:WK + 512], wd[:, WK:WK + 512])
            nc.sync.dma_start(wsb[:, WK + 512:WK + E], wd[:, WK + 512:WK + E])
            nc.sync.dma_start(wsb[:, WQ:WQ + E], wd[:, WQ:WQ + E])
            nc.sync.dma_start(wsb[:, WV:WV + E], wd[:, WV:WV + E])
            for s in range(2, NS):
                nc.sync.dma_start(
                    xs[:, s * 2048:(s + 1) * 2048], xd[:, s * 2048:(s + 1) * 2048])

            def emit_proj_pe(s):
                x0 = s * 2048
                # one PSUM bank per stage (k [0:256], q [256:384],
                # v block 0 [384:512]); v block 1 rides the score-chunk
                # rotation so every pool stays inside the 8-bank budget
                pp = pp_pool.tile([128, 512], F32)
                pk, pq = pp[:, 0:256], pp[:, 256:384]
                spv = sps_pool.tile([128, 512], F32, tag="sp", name="spv")
                pvs = (pp[:, 384:512], spv[:, 0:128])
                for e in range(NE):
                    nc.tensor.matmul(
                        pk, wsb[:, WK + e * 128:WK + (e + 1) * 128],
                        xs[:, x0 + e * 256:x0 + (e + 1) * 256],
                        start=(e == 0), stop=(e == NE - 1))
                for e in range(NE):
                    nc.tensor.matmul(
                        pq, wsb[:, WQ + e * 128:WQ + (e + 1) * 128],
                        xs[:, x0 + e * 256:x0 + e * 256 + 128],
                        start=(e == 0), stop=(e == NE - 1))
                for blk in range(2):
                    for e in range(NE):
                        nc.tensor.matmul(
                            pvs[blk],
                            xs[:, x0 + e * 256 + blk * 128:x0 + e * 256 + (blk + 1) * 128],
                            wsb[:, WV + e * 128:WV + (e + 1) * 128],
                            start=(e == 0), stop=(e == NE - 1))
                return pp, pvs

            def emit_proj_post(s, pp, pp_vs):
                pk, pq = pp[:, 0:256], pp[:, 256:384]
                nc.vector.tensor_scalar_add(
                    kT[:, 2 * s * 128:(2 * s + 2) * 128], pk, bk_sb[:])
                nc.vector.tensor_scalar_add(
                    qT[:, s * 128:(s + 1) * 128], pq, bq_sb[:])
                for blk in range(2):
                    kb = 2 * s + blk
                    nc.vector.tensor_copy(
                        vaug[:, kb * 129:kb * 129 + 128], pp_vs[blk])

            def emit_attn(s):
                n = 2 * s + 2                       # key slots for this stage
                chunks = [(c0, min(c0 + 4, n)) for c0 in range(0, n, 4)]
                C = len(chunks)
                # natural order: the masked chunk is last, its mask-add runs
                # on DVE concurrently with the earlier chunks' exps
                order = list(range(C))
                av = av_pool.tile([128, 129], F32)
                exs = {}

                def emit_score(ci):
                    c0, c1 = chunks[ci]
                    w = (c1 - c0) * 128
                    sp = sps_pool.tile([128, 512], F32, tag="sp", name="sp")
                    for kb in range(c0, c1):
                        m = kb - c0
                        nc.tensor.matmul(
                            sp[:, m * 128:(m + 1) * 128],
                            kT[:, kb * 128:(kb + 1) * 128],
                            qT[:, s * 128:(s + 1) * 128],
                            start=True, stop=True)
                    if c1 == n:  # causal masks live on the last two slots
                        nc.vector.tensor_add(
                            sp[:, w - 256:w], sp[:, w - 256:w], mask_sb[:])
                    ex = ex_pool.tile([128, 512], F16)
                    exs[ci] = ex
                    nc.scalar.activation(
                        ex[:, 0:w], sp[:, 0:w], mybir.ActivationFunctionType.Exp)

                def emit_av(ci, first, last):
                    c0, c1 = chunks[ci]
                    for kb in range(c0, c1):
                        m = kb - c0
                        nc.tensor.matmul(
                            av[:], exs[ci][:, m * 128:(m + 1) * 128],
                            vaug[:, kb * 129:(kb + 1) * 129],
                            start=(first and kb == c0),
                            stop=(last and kb == c1 - 1))

                for ci in order:
                    emit_score(ci)
                for i in range(C - 1):
                    emit_av(order[i], first=(i == 0), last=False)

                def finish():
                    emit_av(order[C - 1], first=(C == 1), last=True)
                    rc = ob_pool.tile([128, 1], F32, tag="rc")
                    nc.vector.reciprocal(rc[:], av[:, 128:129])
                    ob = ob_pool.tile([128, 128], F32, tag="ob")
                    nc.vector.tensor_scalar_mul(ob[:], av[:, 0:128], rc[:])
                    eng = nc.sync if s == NS - 2 else nc.gpsimd
                    eng.dma_start(y[s * 128:(s + 1) * 128, :], ob[:])

                return finish

            # projections run two stages ahead; each stage's final AV chunk
            # is emitted after the next projection's PE work so PE fills the
            # exp wait, while DVE/ACT consumers keep stage-local order
            for s in range(3):
                pp0, vs0 = emit_proj_pe(s)
                emit_proj_post(s, pp0, vs0)
            for s in range(NS - 3):
                finish = emit_attn(s)
                pp2, vs2 = emit_proj_pe(s + 3)
                finish()
                emit_proj_post(s + 3, pp2, vs2)
            fin5 = emit_attn(NS - 3)
            fin5()
            # tail: interleave the two largest stages so stage 6 fills PE
            # during stage 7's exp waits; stage 6 finishes last
            fin7 = emit_attn(NS - 1)
            fin6 = emit_attn(NS - 2)
            fin7()
            fin6()
    nc.compile()
    return nc


def _pack_w(w: np.ndarray) -> np.ndarray:
    # [E, H] -> [128, E]: chunk e at cols e*128, partitions = rows e*128+p
    return w.reshape(NE, 128, H).transpose(1, 0, 2).reshape(128, E)


def _mask(h: int) -> np.ndarray:
    p = np.arange(128)[:, None]  # key position within slot (partition)
    c = np.arange(128)[None, :]  # query position within block (free)
    tri = np.where(p <= c, 0.0, -BIG).astype(np.float32)
    last = np.full((128, 128), -BIG, np.float32) if h == 0 else np.zeros((128, 128), np.float32)
    return np.ascontiguousarray(np.concatenate([tri, last], axis=1))


def kernel(x, Wq, bq, Wk, bk, Wv, bv):
    x = np.asarray(x, dtype=np.float32)
    Wq = np.asarray(Wq, dtype=np.float32)
    Wk = np.asarray(Wk, dtype=np.float32)
    Wv = np.asarray(Wv, dtype=np.float32)
    bq = np.asarray(bq, dtype=np.float32)
    bk = np.asarray(bk, dtype=np.float32)
    bv = np.asarray(bv, dtype=np.float32)

    if "nc" not in _CACHE:
        _CACHE["nc"] = _build()
    nc = _CACHE["nc"]

    scale = 1.0 / math.sqrt(H)
    wsb = np.ascontiguousarray(np.concatenate(
        [_pack_w(Wk), _pack_w(Wq * scale), _pack_w(Wv)], axis=1)).astype(np.float16)
    bq_s = np.ascontiguousarray((bq * scale).reshape(H, 1))
    bk_r = np.ascontiguousarray(bk.reshape(H, 1))
    masks = {0: _mask(0), 1: _mask(1)}

    # [b, blk, pos, e, ep]
    xb = x.astype(np.float16).reshape(B, NB, 128, NE, 128)
    in_maps = []
    for core in range(8):
        b, h = divmod(core, 2)
        own = xb[b, h::2]       # [8, pos, e, ep] blocks h, h+2, ...
        oth = xb[b, 1 - h::2]
        pair = np.stack([own, oth], axis=1)          # [s, which, pos, e, ep]
        xsc = np.ascontiguousarray(
            pair.transpose(4, 0, 3, 1, 2).reshape(128, NE * T))
        in_maps.append({
            "xd": xsc, "wd": wsb, "bqd": bq_s, "bkd": bk_r, "md": masks[h],
        })

    res = run_bass_kernel_spmd(nc, in_maps, core_ids=list(range(8)))
    out = np.empty((B, T, H), dtype=np.float32)
    for core in range(8):
        b, h = divmod(core, 2)
        yc = res.results[core]["y"]
        for s in range(NS):
            g = 2 * s + h
            out[b, g * 128:(g + 1) * 128, :] = yc[s * 128:(s + 1) * 128, :]
    out += bv  # softmax rows sum to 1, so +bv commutes with attention
    return out
